# revision 7
# baseline (speedup 1.0000x reference)
"""Trainium2 Bass kernel v2 for fused multi-head attention (CompositeMHA).

Same sharding/contract as kernel.py (see its docstring): 8 cores =
4 batches x 2 head-halves; per-core [S,E] partial output, host sums pairs.

v2 changes vs baseline:
- Dual DMA queues: weights stream on the Activation-engine HWDGE queue,
  x / biases / outputs on the SP (sync) queue, so weight fetch does not
  serialize behind the 4 MiB x load at startup.
- Projection matmuls run ec-outer across 4 concurrent PSUM chains
  (head x token-half), consuming (xt[ec], w[ec]) DMA pairs in arrival
  order: the PE starts ~1-2 us in instead of waiting ~19 us for the
  whole x/w stream.
- PSUM banks: proj/out-proj chains + softmax-sum rows share a 4-deep
  ring (sums run while proj is quiescent); scores and ctx get 2 each.
- Same-stationary matmuls are emitted back-to-back (token-halves under
  one weight load; fw-blocks under one ctxT chunk) to give the
  legalizer/PE weight-reuse a chance.
"""

import numpy as np
import ml_dtypes

B, S, E, H = 4, 1024, 2048, 16
D = 128
P = 128
HH = 8
EH = HH * D      # 1024 e-columns per half
EC = E // P      # 16 contraction chunks (in-proj)
OC = EH // P     # 8 contraction chunks (out-proj)
KC = S // P      # 8 key chunks
NCORES = 8
BF16 = ml_dtypes.bfloat16

_PROGRAM = None


def _build_program(bench_iters=None, phase="full", opts=None):
    import json
    import os

    opts = dict(opts or {})
    opts.update(json.loads(os.environ.get("KOPT", "{}")))
    import concourse.bass as bass  # noqa: F401
    import concourse.tile as tile
    from concourse import bacc, mybir
    from contextlib import ExitStack

    dt = mybir.dt
    AFT = mybir.ActivationFunctionType

    nc = bacc.Bacc("TRN2", target_bir_lowering=False, debug=False,
                   num_devices=NCORES)

    xT_d = nc.dram_tensor("xT", [E, S], dt.bfloat16, kind="ExternalInput").ap()
    wqT_d = nc.dram_tensor("wqT", [E, EH], dt.bfloat16, kind="ExternalInput").ap()
    wkT_d = nc.dram_tensor("wkT", [E, EH], dt.bfloat16, kind="ExternalInput").ap()
    wvT_d = nc.dram_tensor("wvT", [E, EH], dt.bfloat16, kind="ExternalInput").ap()
    woT_d = nc.dram_tensor("woT", [EH, E], dt.bfloat16, kind="ExternalInput").ap()
    bqT_d = nc.dram_tensor("bqT", [P, HH], dt.float32, kind="ExternalInput").ap()
    bkT_d = nc.dram_tensor("bkT", [P, HH], dt.float32, kind="ExternalInput").ap()
    bv_d = nc.dram_tensor("bv", [1, EH], dt.float32, kind="ExternalInput").ap()
    bo_d = nc.dram_tensor("bo", [1, E], dt.float32, kind="ExternalInput").ap()
    out_d = nc.dram_tensor("out", [S, E], dt.float32, kind="ExternalOutput").ap()

    wq_engine = nc.scalar if opts.get("dual_queue", True) else nc.sync

    with tile.TileContext(nc) as tc, ExitStack() as ctx:
        sb = ctx.enter_context(tc.tile_pool(name="persist", bufs=1))
        wp = ctx.enter_context(tc.tile_pool(name="wstream", bufs=2))
        wop = ctx.enter_context(tc.tile_pool(name="wout", bufs=1))
        ktp = ctx.enter_context(tc.tile_pool(name="ktp", bufs=8))
        qtp = ctx.enter_context(tc.tile_pool(name="qtp", bufs=8))
        ptp = ctx.enter_context(tc.tile_pool(name="ptp",
                                             bufs=opts.get("ptp_bufs", 12)))
        outp = ctx.enter_context(tc.tile_pool(name="outp", bufs=3))
        rowp = ctx.enter_context(tc.tile_pool(name="rowp", bufs=3))
        # PSUM: pp(4: proj/out chains + softmax sums) + sp(2) + cp(2) = 8
        ppp = ctx.enter_context(tc.tile_pool(name="ppsum", bufs=4, space="PSUM"))
        spp = ctx.enter_context(tc.tile_pool(name="spsum", bufs=2, space="PSUM"))
        cpp = ctx.enter_context(tc.tile_pool(name="cpsum", bufs=2, space="PSUM"))

        def emit():
            # ---- persistent loads (x + small biases on the sync queue) ----
            bqt = sb.tile([P, HH], dt.float32, name="bqt", tag="bqt")
            nc.sync.dma_start(bqt[:], bqT_d[:])
            bkt = sb.tile([P, HH], dt.float32, name="bkt", tag="bkt")
            nc.sync.dma_start(bkt[:], bkT_d[:])
            xt = []
            for ec in range(EC):
                t = sb.tile([P, S], dt.bfloat16, name=f"xt{ec}", tag=f"xt{ec}")
                nc.sync.dma_start(t[:], xT_d[ec * P:(ec + 1) * P, :])
                xt.append(t)
            bv_rep = sb.tile([P, EH], dt.float32, name="bv_rep", tag="bv_rep")
            nc.sync.dma_start(bv_rep[:], bv_d.to_broadcast((P, EH)))
            bo_rep = sb.tile([P, E], dt.float32, name="bo_rep", tag="bo_rep")
            nc.sync.dma_start(bo_rep[:], bo_d.to_broadcast((P, E)))
            # all-ones stationary: the sums matmul yields every output row
            # = sum over k, i.e. the softmax denominator pre-broadcast
            ones128 = sb.tile([P, P], dt.bfloat16, name="ones128",
                              tag="ones128")
            nc.vector.memset(ones128[:], 1.0)

            v_sb = []
            for sc in range(KC):
                v_sb.append(sb.tile([P, EH], dt.bfloat16, name=f"v{sc}",
                                    tag=f"v{sc}"))
            ctxT = []
            for h in range(HH):
                ctxT.append(sb.tile([P, S], dt.bfloat16, name=f"ctxT{h}",
                                    tag=f"ctxT{h}"))

            def load_w_tiles(dram, col0, label, nec=EC):
                tiles = []
                for ec in range(nec):
                    t = wp.tile([P, 512], dt.bfloat16,
                                name=f"{label}{ec}", tag=f"w{ec}")
                    wq_engine.dma_start(
                        t[:], dram[ec * P:(ec + 1) * P, col0:col0 + 512])
                    tiles.append(t)
                return tiles

            def proj_kq(w, bias, dest_pool, h0, h1):
                """K^T/Q^T for local heads h0,h1: ec-outer over 4 chains
                (2 heads x 2 token-halves), consuming (xt, w) DMA pairs in
                arrival order at startup."""
                dsts = {}
                ps = {}
                for h in (h0, h1):
                    dsts[h] = dest_pool.tile([P, S], dt.bfloat16,
                                             name=f"kq{h}", tag="kq")
                    for sbl in range(2):
                        ps[h, sbl] = ppp.tile([P, 512], dt.float32,
                                              name=f"pp{h}_{sbl}", tag="pp")
                for ec in range(EC):
                    for h in (h0, h1):
                        hc = (h % 4) * P
                        for sbl in range(2):
                            nc.tensor.matmul(
                                ps[h, sbl][:],
                                w[ec][:, hc:hc + P],
                                xt[ec][:, sbl * 512:(sbl + 1) * 512],
                                start=(ec == 0), stop=(ec == EC - 1))
                for h in (h0, h1):
                    for sbl in range(2):
                        nc.vector.tensor_scalar_add(
                            dsts[h][:, sbl * 512:(sbl + 1) * 512],
                            ps[h, sbl][:], bias[:, h:h + 1])
                return dsts

            # issue every weight load upfront on the ACT HWDGE queue; the
            # wp ring (bufs=2 per tag) paces the actual transfers so each
            # set lands ~one phase ahead of its consumers
            wsets = {}
            for grp in range(2):
                wsets["wk", grp] = load_w_tiles(wkT_d, grp * 512, f"wk{grp}")
                wsets["wq", grp] = load_w_tiles(wqT_d, grp * 512, f"wq{grp}")
                wsets["wv", grp] = load_w_tiles(wvT_d, grp * 512, f"wv{grp}")

            kt = {}
            qt = {}

            for grp in range(2):
                if grp == 1:
                    # wo loads issue here on the ACT queue (after grp0's
                    # exps): all 4 sets resident by out-proj, without
                    # stealing startup HBM bandwidth
                    for fw in range(4):
                        tiles = []
                        for ec in range(OC):
                            t = wop.tile([P, 512], dt.bfloat16,
                                         name=f"wo{fw}_{ec}",
                                         tag=f"wo{fw}_{ec}")
                            wq_engine.dma_start(
                                t[:], woT_d[ec * P:(ec + 1) * P,
                                            fw * 512:(fw + 1) * 512])
                            tiles.append(t)
                        wsets["wo", fw] = tiles
                heads = [grp * 4 + i for i in range(4)]
                wk = wsets["wk", grp]
                kt.update(proj_kq(wk, bkt, ktp, heads[0], heads[1]))
                kt.update(proj_kq(wk, bkt, ktp, heads[2], heads[3]))
                wq = wsets["wq", grp]
                qt.update(proj_kq(wq, bqt, qtp, heads[0], heads[1]))
                qt.update(proj_kq(wq, bqt, qtp, heads[2], heads[3]))

                # V columns for this grp's 4 heads (natural [s, e] layout)
                wv = wsets["wv", grp]
                csl = slice(grp * 512, (grp + 1) * 512)
                for sc in range(0, KC, 2):
                    vps = {s2: ppp.tile([P, 512], dt.float32, name="vps",
                                        tag="pp") for s2 in (sc, sc + 1)}
                    for ec in range(EC):
                        for s2 in (sc, sc + 1):
                            nc.tensor.matmul(
                                vps[s2][:],
                                xt[ec][:, s2 * P:(s2 + 1) * P],
                                wv[ec][:],
                                start=(ec == 0), stop=(ec == EC - 1))
                    for s2 in (sc, sc + 1):
                        nc.vector.tensor_add(
                            v_sb[s2][:, csl], vps[s2][:], bv_rep[:, csl])

                # attention for this grp's heads
                lag = opts.get("pipe_lag", 3)
                no_epi = opts.get("no_epi", False)
                no_sums = opts.get("no_sums", False) or no_epi
                act_fn = {"exp": AFT.Exp, "copy": AFT.Copy,
                          "ident": AFT.Identity}[opts.get("act", "exp")]
                use_act = opts.get("use_act", True)
                gp_bcast = opts.get("gp_bcast", True)
                for h in (heads if phase != "proj" else []):
                    for qb in range(2):
                        qsl = slice(qb * 512, (qb + 1) * 512)
                        su = ppp.tile([P, 512], dt.float32,
                                      name=f"su{h}_{qb}", tag="pp")
                        cp = cpp.tile([P, 512], dt.float32,
                                      name=f"cp{h}_{qb}", tag="cp")
                        pts = {}
                        # pass 1: scores + exp (PE/ACT pipelined)
                        for kc in range(KC):
                            sps = spp.tile([P, 512], dt.float32, name="sps",
                                           tag="sp")
                            nc.tensor.matmul(sps[:],
                                             kt[h][:, kc * P:(kc + 1) * P],
                                             qt[h][:, qsl],
                                             start=True, stop=True)
                            pt = ptp.tile([P, 512], dt.bfloat16, name="pt",
                                          tag="pt")
                            if use_act:
                                nc.scalar.activation(pt[:], sps[:], act_fn)
                            else:
                                nc.vector.tensor_copy(pt[:], sps[:])
                            pts[kc] = pt
                        # pass 2: sums chain (every output row = sum over k
                        # = the softmax denominator, pre-broadcast)
                        if not no_sums:
                            for kc in range(KC):
                                nc.tensor.matmul(su[:], ones128[:],
                                                 pts[kc][:],
                                                 start=(kc == 0),
                                                 stop=(kc == KC - 1))
                        # pass 3: ctx chain
                        for kc in range(KC):
                            nc.tensor.matmul(
                                cp[:],
                                v_sb[kc][:, h * P:(h + 1) * P],
                                pts[kc][:], start=(kc == 0),
                                stop=(kc == KC - 1))
                        if no_epi:
                            nc.vector.tensor_copy(ctxT[h][:, qsl], cp[:])
                            continue
                        rep = rowp.tile([P, 512], dt.float32, name="rep",
                                        tag="rep")
                        if no_sums or opts.get("no_recip", False):
                            nc.vector.memset(rep[:], 1.0)
                        else:
                            nc.vector.reciprocal(rep[:], su[:])
                        nc.vector.tensor_mul(ctxT[h][:, qsl], cp[:], rep[:])

            # ---- output projection (partial: contracts this e-half) ----
            # qc outer / fw inner: consecutive chains stream 32 distinct wo
            # tiles before any repeats
            for qc in range(KC if phase == "full" else 0):
                for fw in range(4):
                    wo = wsets["wo", fw]
                    ps = ppp.tile([P, 512], dt.float32, name="ops", tag="pp")
                    for h in range(HH):
                        nc.tensor.matmul(
                            ps[:],
                            ctxT[h][:, qc * P:(qc + 1) * P],
                            wo[h][:],
                            start=(h == 0), stop=(h == HH - 1))
                    osb = outp.tile([P, 512], dt.float32, name="osb",
                                    tag="ot")
                    nc.vector.tensor_add(
                        osb[:], ps[:], bo_rep[:, fw * 512:(fw + 1) * 512])
                    nc.sync.dma_start(
                        out_d[qc * P:(qc + 1) * P,
                              fw * 512:(fw + 1) * 512], osb[:])

        if bench_iters is None:
            emit()
        else:
            with tc.For_i(0, bench_iters, 1):
                emit()

    nc.compile()
    return nc


def _get_program():
    global _PROGRAM
    if _PROGRAM is None:
        _PROGRAM = _build_program()
    return _PROGRAM


def make_in_maps(query, in_proj_weight, in_proj_bias, out_proj_weight,
                 out_proj_bias):
    """Host-side sharding: slice/transpose/cast per core. Pure layout prep."""
    x = np.asarray(query, dtype=np.float32)
    W = np.asarray(in_proj_weight, dtype=np.float32)
    b = np.asarray(in_proj_bias, dtype=np.float32)
    Wo = np.asarray(out_proj_weight, dtype=np.float32)
    bo = np.asarray(out_proj_bias, dtype=np.float32)

    sc = np.float32(1.0 / np.sqrt(D))
    wqT = np.ascontiguousarray((W[:E] * sc).T).astype(BF16)       # [E, E]
    wkT = np.ascontiguousarray(W[E:2 * E].T).astype(BF16)
    wvT = np.ascontiguousarray(W[2 * E:].T).astype(BF16)
    woT = np.ascontiguousarray(Wo.T).astype(BF16)                 # [E, E]
    bq_s = (b[:E] * sc).reshape(H, P)
    bk_s = b[E:2 * E].reshape(H, P)
    bv_s = b[2 * E:].reshape(1, E)
    bo_r = np.ascontiguousarray(bo.reshape(1, E))
    bo_zero = np.zeros_like(bo_r)

    in_maps = []
    for c in range(NCORES):
        bi, hh = c // 2, c % 2
        esl = slice(hh * EH, (hh + 1) * EH)
        xT = np.ascontiguousarray(x[bi].T).astype(BF16)
        in_maps.append({
            "xT": xT,
            "wqT": np.ascontiguousarray(wqT[:, esl]),
            "wkT": np.ascontiguousarray(wkT[:, esl]),
            "wvT": np.ascontiguousarray(wvT[:, esl]),
            "woT": np.ascontiguousarray(woT[esl, :]),
            "bqT": np.ascontiguousarray(bq_s[hh * HH:(hh + 1) * HH].T),
            "bkT": np.ascontiguousarray(bk_s[hh * HH:(hh + 1) * HH].T),
            "bv": np.ascontiguousarray(bv_s[:, esl]),
            "bo": bo_r if hh == 0 else bo_zero,
        })
    return in_maps


def assemble_out(results):
    """Gather: sum each batch's two tensor-parallel partial outputs."""
    out = np.empty((B, S, E), dtype=np.float32)
    for bi in range(B):
        out[bi] = results[2 * bi]["out"] + results[2 * bi + 1]["out"]
    return out


def kernel(query, in_proj_weight, in_proj_bias, out_proj_weight,
           out_proj_bias):
    from concourse import bass_utils
    nc = _get_program()
    in_maps = make_in_maps(query, in_proj_weight, in_proj_bias,
                           out_proj_weight, out_proj_bias)
    res = bass_utils.run_bass_kernel_spmd(nc, in_maps,
                                          core_ids=list(range(NCORES)))
    return assemble_out(res.results)


# revision 8
# speedup vs baseline: 1.0374x; 1.0374x over previous
"""Trainium2 Bass kernel v2 for fused multi-head attention (CompositeMHA).

Same sharding/contract as kernel.py (see its docstring): 8 cores =
4 batches x 2 head-halves; per-core [S,E] partial output, host sums pairs.

v2 changes vs baseline:
- Dual DMA queues: weights stream on the Activation-engine HWDGE queue,
  x / biases / outputs on the SP (sync) queue, so weight fetch does not
  serialize behind the 4 MiB x load at startup.
- Projection matmuls run ec-outer across 4 concurrent PSUM chains
  (head x token-half), consuming (xt[ec], w[ec]) DMA pairs in arrival
  order: the PE starts ~1-2 us in instead of waiting ~19 us for the
  whole x/w stream.
- PSUM banks: proj/out-proj chains + softmax-sum rows share a 4-deep
  ring (sums run while proj is quiescent); scores and ctx get 2 each.
- Same-stationary matmuls are emitted back-to-back (token-halves under
  one weight load; fw-blocks under one ctxT chunk) to give the
  legalizer/PE weight-reuse a chance.
"""

import numpy as np
import ml_dtypes

B, S, E, H = 4, 1024, 2048, 16
D = 128
P = 128
HH = 8
EH = HH * D      # 1024 e-columns per half
EC = E // P      # 16 contraction chunks (in-proj)
OC = EH // P     # 8 contraction chunks (out-proj)
KC = S // P      # 8 key chunks
NCORES = 8
BF16 = ml_dtypes.bfloat16

_PROGRAM = None


def _build_program(bench_iters=None, phase="full", opts=None):
    import json
    import os

    opts = dict(opts or {})
    opts.update(json.loads(os.environ.get("KOPT", "{}")))
    import concourse.bass as bass  # noqa: F401
    import concourse.tile as tile
    from concourse import bacc, mybir
    from contextlib import ExitStack

    dt = mybir.dt
    AFT = mybir.ActivationFunctionType

    nc = bacc.Bacc("TRN2", target_bir_lowering=False, debug=False,
                   num_devices=NCORES)

    xT_d = nc.dram_tensor("xT", [E, S], dt.bfloat16, kind="ExternalInput").ap()
    wqT_d = nc.dram_tensor("wqT", [E, EH], dt.bfloat16, kind="ExternalInput").ap()
    wkT_d = nc.dram_tensor("wkT", [E, EH], dt.bfloat16, kind="ExternalInput").ap()
    wvT_d = nc.dram_tensor("wvT", [E, EH], dt.bfloat16, kind="ExternalInput").ap()
    woT_d = nc.dram_tensor("woT", [EH, E], dt.bfloat16, kind="ExternalInput").ap()
    bqT_d = nc.dram_tensor("bqT", [P, HH], dt.float32, kind="ExternalInput").ap()
    bkT_d = nc.dram_tensor("bkT", [P, HH], dt.float32, kind="ExternalInput").ap()
    bv_d = nc.dram_tensor("bv", [1, EH], dt.float32, kind="ExternalInput").ap()
    bo_d = nc.dram_tensor("bo", [1, E], dt.float32, kind="ExternalInput").ap()
    out_d = nc.dram_tensor("out", [S, E], dt.float32, kind="ExternalOutput").ap()

    wq_engine = nc.scalar if opts.get("dual_queue", True) else nc.sync

    with tile.TileContext(nc) as tc, ExitStack() as ctx:
        sb = ctx.enter_context(tc.tile_pool(name="persist", bufs=1))
        wp = ctx.enter_context(tc.tile_pool(name="wstream", bufs=2))
        wop = ctx.enter_context(tc.tile_pool(name="wout", bufs=1))
        ktp = ctx.enter_context(tc.tile_pool(name="ktp", bufs=8))
        qtp = ctx.enter_context(tc.tile_pool(name="qtp", bufs=8))
        ptp = ctx.enter_context(tc.tile_pool(name="ptp",
                                             bufs=opts.get("ptp_bufs", 12)))
        outp = ctx.enter_context(tc.tile_pool(name="outp", bufs=3))
        rowp = ctx.enter_context(tc.tile_pool(name="rowp", bufs=2))
        # PSUM: pp(4: proj/out chains + softmax sums) + sp(2) + cp(2) = 8
        ppp = ctx.enter_context(tc.tile_pool(name="ppsum", bufs=4, space="PSUM"))
        spp = ctx.enter_context(tc.tile_pool(name="spsum", bufs=2, space="PSUM"))
        cpp = ctx.enter_context(tc.tile_pool(name="cpsum", bufs=2, space="PSUM"))

        def emit():
            # ---- persistent loads (x + small biases on the sync queue) ----
            bqt = sb.tile([P, HH], dt.float32, name="bqt", tag="bqt")
            nc.sync.dma_start(bqt[:], bqT_d[:])
            bkt = sb.tile([P, HH], dt.float32, name="bkt", tag="bkt")
            nc.sync.dma_start(bkt[:], bkT_d[:])
            xt = []
            for ec in range(EC):
                t = sb.tile([P, S], dt.bfloat16, name=f"xt{ec}", tag=f"xt{ec}")
                nc.sync.dma_start(t[:], xT_d[ec * P:(ec + 1) * P, :])
                xt.append(t)
            bv_rep = sb.tile([P, EH], dt.float32, name="bv_rep", tag="bv_rep")
            nc.sync.dma_start(bv_rep[:], bv_d.to_broadcast((P, EH)))
            bo_rep = sb.tile([P, E], dt.float32, name="bo_rep", tag="bo_rep")
            nc.sync.dma_start(bo_rep[:], bo_d.to_broadcast((P, E)))
            # all-ones stationary: the sums matmul yields every output row
            # = sum over k, i.e. the softmax denominator pre-broadcast
            ones128 = sb.tile([P, P], dt.bfloat16, name="ones128",
                              tag="ones128")
            nc.vector.memset(ones128[:], 1.0)

            v_sb = []
            for sc in range(KC):
                v_sb.append(sb.tile([P, EH], dt.bfloat16, name=f"v{sc}",
                                    tag=f"v{sc}"))
            ctxT = []
            for h in range(HH):
                ctxT.append(sb.tile([P, S], dt.bfloat16, name=f"ctxT{h}",
                                    tag=f"ctxT{h}"))

            def load_w_tiles(dram, col0, label, nec=EC):
                tiles = []
                for ec in range(nec):
                    t = wp.tile([P, 512], dt.bfloat16,
                                name=f"{label}{ec}", tag=f"w{ec}")
                    wq_engine.dma_start(
                        t[:], dram[ec * P:(ec + 1) * P, col0:col0 + 512])
                    tiles.append(t)
                return tiles

            def proj_kq(w, bias, dest_pool, h0, h1):
                """K^T/Q^T for local heads h0,h1: ec-outer over 4 chains
                (2 heads x 2 token-halves), consuming (xt, w) DMA pairs in
                arrival order at startup."""
                dsts = {}
                ps = {}
                for h in (h0, h1):
                    dsts[h] = dest_pool.tile([P, S], dt.bfloat16,
                                             name=f"kq{h}", tag="kq")
                    for sbl in range(2):
                        ps[h, sbl] = ppp.tile([P, 512], dt.float32,
                                              name=f"pp{h}_{sbl}", tag="pp")
                for ec in range(EC):
                    for h in (h0, h1):
                        hc = (h % 4) * P
                        for sbl in range(2):
                            nc.tensor.matmul(
                                ps[h, sbl][:],
                                w[ec][:, hc:hc + P],
                                xt[ec][:, sbl * 512:(sbl + 1) * 512],
                                start=(ec == 0), stop=(ec == EC - 1))
                for h in (h0, h1):
                    for sbl in range(2):
                        nc.vector.tensor_scalar_add(
                            dsts[h][:, sbl * 512:(sbl + 1) * 512],
                            ps[h, sbl][:], bias[:, h:h + 1])
                return dsts

            # issue every weight load upfront on the ACT HWDGE queue; the
            # wp ring (bufs=2 per tag) paces the actual transfers so each
            # set lands ~one phase ahead of its consumers
            wsets = {}
            for grp in range(2):
                wsets["wk", grp] = load_w_tiles(wkT_d, grp * 512, f"wk{grp}")
                wsets["wq", grp] = load_w_tiles(wqT_d, grp * 512, f"wq{grp}")
                wsets["wv", grp] = load_w_tiles(wvT_d, grp * 512, f"wv{grp}")

            kt = {}
            qt = {}

            for grp in range(2):
                if grp == 1:
                    # wo loads issue here on the ACT queue (after grp0's
                    # exps): all 4 sets resident by out-proj, without
                    # stealing startup HBM bandwidth
                    for fw in range(4):
                        tiles = []
                        for ec in range(OC):
                            t = wop.tile([P, 512], dt.bfloat16,
                                         name=f"wo{fw}_{ec}",
                                         tag=f"wo{fw}_{ec}")
                            wq_engine.dma_start(
                                t[:], woT_d[ec * P:(ec + 1) * P,
                                            fw * 512:(fw + 1) * 512])
                            tiles.append(t)
                        wsets["wo", fw] = tiles
                heads = [grp * 4 + i for i in range(4)]
                wk = wsets["wk", grp]
                kt.update(proj_kq(wk, bkt, ktp, heads[0], heads[1]))
                kt.update(proj_kq(wk, bkt, ktp, heads[2], heads[3]))
                wq = wsets["wq", grp]
                qt.update(proj_kq(wq, bqt, qtp, heads[0], heads[1]))
                qt.update(proj_kq(wq, bqt, qtp, heads[2], heads[3]))

                # V columns for this grp's 4 heads (natural [s, e] layout)
                wv = wsets["wv", grp]
                csl = slice(grp * 512, (grp + 1) * 512)
                for sc in range(0, KC, 2):
                    vps = {s2: ppp.tile([P, 512], dt.float32, name="vps",
                                        tag="pp") for s2 in (sc, sc + 1)}
                    for ec in range(EC):
                        for s2 in (sc, sc + 1):
                            nc.tensor.matmul(
                                vps[s2][:],
                                xt[ec][:, s2 * P:(s2 + 1) * P],
                                wv[ec][:],
                                start=(ec == 0), stop=(ec == EC - 1))
                    for s2 in (sc, sc + 1):
                        nc.vector.tensor_add(
                            v_sb[s2][:, csl], vps[s2][:], bv_rep[:, csl])

                # attention for this grp's heads
                lag = opts.get("pipe_lag", 3)
                no_epi = opts.get("no_epi", False)
                no_sums = opts.get("no_sums", False) or no_epi
                act_fn = {"exp": AFT.Exp, "copy": AFT.Copy,
                          "ident": AFT.Identity}[opts.get("act", "exp")]
                use_act = opts.get("use_act", True)
                gp_bcast = opts.get("gp_bcast", True)
                for h in (heads if phase != "proj" else []):
                    for qb in range(2):
                        qsl = slice(qb * 512, (qb + 1) * 512)
                        su = ppp.tile([P, 512], dt.float32,
                                      name=f"su{h}_{qb}", tag="pp")
                        cp = cpp.tile([P, 512], dt.float32,
                                      name=f"cp{h}_{qb}", tag="cp")
                        pts = {}
                        # pass 1: scores + exp (PE/ACT pipelined)
                        for kc in range(KC):
                            sps = spp.tile([P, 512], dt.float32, name="sps",
                                           tag="sp")
                            nc.tensor.matmul(sps[:],
                                             kt[h][:, kc * P:(kc + 1) * P],
                                             qt[h][:, qsl],
                                             start=True, stop=True)
                            pt = ptp.tile([P, 512], dt.bfloat16, name="pt",
                                          tag="pt")
                            if use_act:
                                nc.scalar.activation(pt[:], sps[:], act_fn)
                            else:
                                nc.vector.tensor_copy(pt[:], sps[:])
                            pts[kc] = pt
                        # pass 2: sums chain (every output row = sum over k
                        # = the softmax denominator, pre-broadcast)
                        if not no_sums:
                            for kc in range(KC):
                                nc.tensor.matmul(su[:], ones128[:],
                                                 pts[kc][:],
                                                 start=(kc == 0),
                                                 stop=(kc == KC - 1))
                        # pass 3: ctx chain
                        for kc in range(KC):
                            nc.tensor.matmul(
                                cp[:],
                                v_sb[kc][:, h * P:(h + 1) * P],
                                pts[kc][:], start=(kc == 0),
                                stop=(kc == KC - 1))
                        if no_epi:
                            nc.vector.tensor_copy(ctxT[h][:, qsl], cp[:])
                            continue
                        rep = rowp.tile([P, 512], dt.float32, name="rep",
                                        tag="rep")
                        if no_sums or opts.get("no_recip", False):
                            nc.vector.memset(rep[:], 1.0)
                        else:
                            nc.vector.reciprocal(rep[:], su[:])
                        nc.vector.tensor_mul(ctxT[h][:, qsl], cp[:], rep[:])

            # ---- output projection (partial: contracts this e-half) ----
            # qc outer / fw inner: consecutive chains stream 32 distinct wo
            # tiles before any repeats
            for qc in range(KC if phase == "full" else 0):
                for fw in range(4):
                    wo = wsets["wo", fw]
                    ps = ppp.tile([P, 512], dt.float32, name="ops", tag="pp")
                    for h in range(HH):
                        nc.tensor.matmul(
                            ps[:],
                            ctxT[h][:, qc * P:(qc + 1) * P],
                            wo[h][:],
                            start=(h == 0), stop=(h == HH - 1))
                    osb = outp.tile([P, 512], dt.float32, name="osb",
                                    tag="ot")
                    nc.vector.tensor_add(
                        osb[:], ps[:], bo_rep[:, fw * 512:(fw + 1) * 512])
                    nc.sync.dma_start(
                        out_d[qc * P:(qc + 1) * P,
                              fw * 512:(fw + 1) * 512], osb[:])

        if bench_iters is None:
            emit()
        else:
            with tc.For_i(0, bench_iters, 1):
                emit()

    nc.compile()
    return nc


def _get_program():
    global _PROGRAM
    if _PROGRAM is None:
        _PROGRAM = _build_program()
    return _PROGRAM


def make_in_maps(query, in_proj_weight, in_proj_bias, out_proj_weight,
                 out_proj_bias):
    """Host-side sharding: slice/transpose/cast per core. Pure layout prep."""
    x = np.asarray(query, dtype=np.float32)
    W = np.asarray(in_proj_weight, dtype=np.float32)
    b = np.asarray(in_proj_bias, dtype=np.float32)
    Wo = np.asarray(out_proj_weight, dtype=np.float32)
    bo = np.asarray(out_proj_bias, dtype=np.float32)

    sc = np.float32(1.0 / np.sqrt(D))
    wqT = np.ascontiguousarray((W[:E] * sc).T).astype(BF16)       # [E, E]
    wkT = np.ascontiguousarray(W[E:2 * E].T).astype(BF16)
    wvT = np.ascontiguousarray(W[2 * E:].T).astype(BF16)
    woT = np.ascontiguousarray(Wo.T).astype(BF16)                 # [E, E]
    bq_s = (b[:E] * sc).reshape(H, P)
    bk_s = b[E:2 * E].reshape(H, P)
    bv_s = b[2 * E:].reshape(1, E)
    bo_r = np.ascontiguousarray(bo.reshape(1, E))
    bo_zero = np.zeros_like(bo_r)

    in_maps = []
    for c in range(NCORES):
        bi, hh = c // 2, c % 2
        esl = slice(hh * EH, (hh + 1) * EH)
        xT = np.ascontiguousarray(x[bi].T).astype(BF16)
        in_maps.append({
            "xT": xT,
            "wqT": np.ascontiguousarray(wqT[:, esl]),
            "wkT": np.ascontiguousarray(wkT[:, esl]),
            "wvT": np.ascontiguousarray(wvT[:, esl]),
            "woT": np.ascontiguousarray(woT[esl, :]),
            "bqT": np.ascontiguousarray(bq_s[hh * HH:(hh + 1) * HH].T),
            "bkT": np.ascontiguousarray(bk_s[hh * HH:(hh + 1) * HH].T),
            "bv": np.ascontiguousarray(bv_s[:, esl]),
            "bo": bo_r if hh == 0 else bo_zero,
        })
    return in_maps


def assemble_out(results):
    """Gather: sum each batch's two tensor-parallel partial outputs."""
    out = np.empty((B, S, E), dtype=np.float32)
    for bi in range(B):
        out[bi] = results[2 * bi]["out"] + results[2 * bi + 1]["out"]
    return out


def kernel(query, in_proj_weight, in_proj_bias, out_proj_weight,
           out_proj_bias):
    from concourse import bass_utils
    nc = _get_program()
    in_maps = make_in_maps(query, in_proj_weight, in_proj_bias,
                           out_proj_weight, out_proj_bias)
    res = bass_utils.run_bass_kernel_spmd(nc, in_maps,
                                          core_ids=list(range(NCORES)))
    return assemble_out(res.results)
